# revision 19
# baseline (speedup 1.0000x reference)
"""Trainium2 Bass kernel for fused linear cross-attention + 1x1 conv + LayerNorm.

Computation (per batch element b, N=4096 tokens, D=512 channels, H=8 heads):
    kq = x2[b].T viewed as [H, 64, N]; v = x1[b].T viewed as [H, 64, N]
    key   = softmax(kq over N);  query = softmax(kq over head-channels)
    context  = key @ v.T     [H, 64, 64]
    attended = context.T @ query  -> agg [512, N]
    y = conv_w @ agg + conv_b    -> [N, 1024]
    out = LayerNorm(y) * ln_w + ln_b

Sharding: pure data-parallel over batch B=8 across the 8 NeuronCores (one
batch element per core, no collectives).

Kernel-level choices (v4):
  - softmax without max-subtraction (inputs are unit-normal; exp is safe) so
    key/query share one exp(x2) pass.
  - bf16 matmul operands; fp8 DoubleRow rejected (e4m3 noise ~3% > gate).
  - xmix host layout [x2 | 4 x (ones2 | x1_block)]: context matmuls stream
    130-col windows; key-softmax denominators ride the ones columns.
  - conv bias folded into MT (query softmax sums to 1 per head -> sum_k q = 8,
    so MT += conv_b/8); no bias matmuls.
  - phase 1 processes token PAIRS (256 tokens per elementwise op) to halve
    instruction + semaphore overhead: the engines run ~1 op each per 256
    tokens (exp on scalar, reduce+recip on vector, q-normalize on gpsimd,
    qcm evacuation alternating scalar/vector, one chunk-pair delayed).
  - conv: 8x512 matmuls per tile at the 215ns streaming roofline, e-outer
    so each PSUM half completes early; bn_stats of half 0 overlaps the
    half-1 matmuls; per-half normalize releases PSUM banks sooner.
  - output DMA'd as bf16 and upcast on host (error budget ~8x under gate).
"""

import numpy as np

B, N, D = 8, 4096, 512
HEADS = 8
HK = D // HEADS  # 64
E2 = 2 * D  # 1024
NT = N // 128  # 32 token tiles
NP = NT // 2  # 16 token pairs
WIN = 130  # per-block context window: 2 ones cols + 128 x1 cols
XW = D + 4 * WIN  # 1032
LN_EPS = 1e-5

_CACHE = {}


def _build(apply_ln_affine: bool):
    import concourse.bacc as bacc
    import concourse.mybir as mybir
    import concourse.tile as tile
    import concourse.bass as bass
    from concourse.masks import make_identity

    f32 = mybir.dt.float32
    bf16 = mybir.dt.bfloat16
    AF = mybir.ActivationFunctionType
    ALU = mybir.AluOpType
    AX = mybir.AxisListType

    nc = bacc.Bacc("TRN2", target_bir_lowering=False, debug=False)

    xmixd = nc.dram_tensor("xmix", [N, XW], bf16, kind="ExternalInput")
    cwTd = nc.dram_tensor("convT", [D, E2], bf16, kind="ExternalInput")
    cb8d = nc.dram_tensor("convb8", [1, E2], bf16, kind="ExternalInput")
    if apply_ln_affine:
        lnwd = nc.dram_tensor("lnw", [1, E2], f32, kind="ExternalInput")
        lnbd = nc.dram_tensor("lnb", [1, E2], f32, kind="ExternalInput")
    outd = nc.dram_tensor("out", [N, E2], bf16, kind="ExternalOutput")

    def bcast_row(src):
        return bass.AP(
            tensor=src.tensor, offset=src.offset,
            ap=[[0, 128]] + list(src.ap)[1:],
        )

    with tile.TileContext(nc) as tc:
        with (
            tc.tile_pool(name="consts", bufs=1) as consts,
            tc.tile_pool(name="resident", bufs=1) as res,
            tc.tile_pool(name="small", bufs=8) as small,
            tc.tile_pool(name="xstream", bufs=4) as xs,
            tc.tile_pool(name="qstream", bufs=3) as qs,
            tc.tile_pool(name="outs", bufs=3) as outs,
        ):
            ident = consts.tile([128, 128], bf16, tag="ident", name="ident")
            make_identity(nc, ident[:])
            ones = consts.tile([128, 128], bf16, tag="ones", name="ones")
            nc.gpsimd.memset(ones[:], 1.0)
            eps_t = consts.tile([128, 1], f32, tag="eps", name="eps")
            nc.gpsimd.memset(eps_t[:], LN_EPS)
            cwT = [consts.tile([128, E2], bf16, tag=f"cwT{j}", name=f"cwT{j}")
                   for j in range(4)]
            cbb8 = consts.tile([128, E2], bf16, tag="cbb8", name="cbb8")
            if apply_ln_affine:
                lnw_b = consts.tile([128, E2], f32, tag="lnw", name="lnw")
                lnb_b = consts.tile([128, E2], f32, tag="lnb", name="lnb")

            qcm = res.tile([128, 4, N], bf16, tag="qcm", name="qcm")

            # ---- Phase 1: exp, query softmax + transpose, context accumulation
            with tc.tile_pool(name="ph1psum", bufs=1, space="PSUM") as c0pool, \
                 tc.tile_pool(name="qtpsum", bufs=4, space="PSUM") as qtp:
                c0 = [c0pool.tile([128, WIN], f32, tag=f"c0_{p}", name=f"c0_{p}")
                      for p in range(4)]

                qts = {}

                def evac_qcm(g, force_scalar=False):
                    ptok = slice(g * 256, (g + 1) * 256)
                    src = qts[g][:].rearrange("p c (j n) -> p j c n", j=4)
                    if force_scalar or g % 2 == 0:
                        nc.scalar.copy(out=qcm[:, :, ptok].rearrange(
                            "p j (c n) -> p j c n", c=2), in_=src)
                    else:
                        nc.vector.tensor_copy(out=qcm[:, :, ptok].rearrange(
                            "p j (c n) -> p j c n", c=2), in_=src)
                    del qts[g]

                for g in range(NP):
                    xm = xs.tile([128, 2, XW], bf16, tag="xm", name="xm")
                    nc.sync.dma_start(
                        out=xm[:],
                        in_=xmixd[g * 256:(g + 1) * 256, :].rearrange(
                            "(c p) w -> p c w", p=128),
                    )
                    E = xs.tile([128, 2, D], bf16, tag="E", name="E")
                    nc.scalar.activation(E[:], xm[:, :, 0:D], AF.Exp)

                    # stage weights one transfer per pair: spread out so they
                    # never contend with the xmix stream; done long before
                    # the MT build needs them.
                    if 2 <= g <= 5:
                        j = g - 2
                        nc.gpsimd.dma_start(
                            out=cwT[j][:], in_=cwTd[j * 128:(j + 1) * 128, :])
                    elif g == 6:
                        nc.gpsimd.dma_start(out=cbb8[:], in_=bcast_row(cb8d[:, :]))
                        if apply_ln_affine:
                            nc.gpsimd.dma_start(out=lnw_b[:], in_=bcast_row(lnwd[:, :]))
                            nc.gpsimd.dma_start(out=lnb_b[:], in_=bcast_row(lnbd[:, :]))

                    for cc in range(2):
                        c = 2 * g + cc
                        for p in range(4):
                            win = xm[:, cc, D + p * WIN:D + (p + 1) * WIN]
                            nc.tensor.matmul(
                                c0[p][:, :], E[:, cc, p * 128:(p + 1) * 128], win,
                                start=(c == 0), stop=(c == NT - 1),
                            )

                    cs = small.tile([128, 16], f32, tag="cs", name="cs")
                    nc.vector.tensor_reduce(
                        cs[:], E[:].rearrange("p c (h k) -> p (c h) k", h=HEADS),
                        axis=AX.X, op=ALU.add,
                    )
                    R = small.tile([128, 16], f32, tag="R", name="R")
                    nc.vector.reciprocal(R[:], cs[:])

                    q = qs.tile([128, 2, D], bf16, tag="q", name="q")
                    nc.gpsimd.tensor_tensor(
                        out=q[:].rearrange("p c (h k) -> p (c h) k", h=HEADS),
                        in0=E[:].rearrange("p c (h k) -> p (c h) k", h=HEADS),
                        in1=R[:].unsqueeze(2).broadcast_to((128, 16, HK)),
                        op=ALU.mult,
                    )

                    qt = qtp.tile([128, 2, 512], bf16, tag="qt", name="qt")
                    for cc in range(2):
                        for j in range(4):
                            nc.tensor.transpose(
                                qt[:, cc, j * 128:(j + 1) * 128],
                                q[:, cc, j * 128:(j + 1) * 128], ident[:],
                            )
                    qts[g] = qt

                    # evacuate two pairs behind: the transposes it reads are
                    # long finished, so the copy never stalls its queue.
                    if g >= 2:
                        evac_qcm(g - 2)

                # both on scalar: vector goes straight to the A extraction
                evac_qcm(NP - 2, force_scalar=True)
                evac_qcm(NP - 1, force_scalar=True)

                # ---- context normalization -> block-diagonal A
                A = [res.tile([128, 128], bf16, tag=f"A{p}", name=f"A{p}")
                     for p in range(4)]
                for p in range(4):
                    rec = small.tile([128, 1], f32, tag="rrec", name="rrec")
                    nc.vector.reciprocal(rec[:], c0[p][:, 0:1])
                    nc.gpsimd.memset(A[p][:], 0.0)
                    for i in range(2):
                        ks = slice(i * 64, (i + 1) * 64)
                        nc.vector.tensor_scalar_mul(
                            out=A[p][ks, i * 64:(i + 1) * 64],
                            in0=c0[p][ks, 2 + i * 64:2 + (i + 1) * 64],
                            scalar1=rec[ks, :],
                        )

            # ---- Fuse attended + conv bias into MT[p] = A[p].T-trans @ cwT[p]
            # + conv_b/8 (query softmax rows sum to 1 per head, 8 heads).
            AT = [res.tile([128, 128], bf16, tag=f"AT{p}", name=f"AT{p}")
                  for p in range(4)]
            MT = [res.tile([128, E2], bf16, tag=f"MT{p}", name=f"MT{p}")
                  for p in range(4)]
            with tc.tile_pool(name="atpsum", bufs=2, space="PSUM") as atp, \
                 tc.tile_pool(name="mpsum", bufs=2, space="PSUM") as mp, \
                 tc.tile_pool(name="warm", bufs=1, space="PSUM") as wp:
                # a few filler matmuls keep the PE activity window hot across
                # the phase boundary so conv starts at full clock (HAM);
                # results are discarded.
                wt = wp.tile([128, 512], f32, tag="wt", name="wt")

                def warm(k):
                    for _ in range(k):
                        nc.tensor.matmul(wt[:, :], ident[:], cwT[0][:, 0:512])

                warm(3)
                for p in range(4):
                    atps = atp.tile([128, 128], bf16, tag="atps", name="atps")
                    nc.tensor.transpose(atps[:], A[p][:], ident[:])
                    nc.scalar.copy(out=AT[p][:], in_=atps[:])
                warm(3)
                for p in range(4):
                    mps = mp.tile([128, E2], f32, tag="mps", name="mps")
                    for e in range(2):
                        es = slice(e * 512, (e + 1) * 512)
                        nc.tensor.matmul(mps[:, es], AT[p][:], cwT[p][:, es],
                                         start=True, stop=False)
                        # bias on the PE: sum_v ones = 128, and cbb8 holds
                        # conv_b/1024, so this accumulates exactly conv_b/8.
                        nc.tensor.matmul(mps[:, es], ones[:], cbb8[:, es],
                                         start=False, stop=True)
                    if p % 2 == 0:
                        nc.scalar.copy(out=MT[p][:], in_=mps[:])
                    else:
                        nc.vector.tensor_copy(out=MT[p][:], in_=mps[:])

            # ---- conv (+folded bias) + LayerNorm
            with tc.tile_pool(name="ypsum", bufs=4, space="PSUM") as yp:
                for t in range(NT):
                    tok = slice(t * 128, (t + 1) * 128)
                    y = yp.tile([128, E2], f32, tag="y", name="y")
                    stats = small.tile([128, 2, 6], f32, tag="stats", name="stats")
                    # e-outer: finish PSUM half 0, start its stats while the
                    # half-1 matmuls still stream.
                    for e in range(2):
                        es = slice(e * 512, (e + 1) * 512)
                        for j in range(4):
                            nc.tensor.matmul(
                                y[:, es], qcm[:, j, tok], MT[j][:, es],
                                start=(j == 0), stop=(j == 3),
                            )
                        nc.vector.bn_stats(stats[:, e, :], y[:, es])

                    mv = small.tile([128, 2], f32, tag="mv", name="mv")
                    nc.vector.bn_aggr(mv[:], stats[:])
                    sd = small.tile([128, 1], f32, tag="sd", name="sd")
                    nc.scalar.activation(sd[:], mv[:, 1:2], AF.Sqrt, bias=eps_t[:])
                    rr = small.tile([128, 1], f32, tag="rr", name="rr")
                    nc.vector.reciprocal(rr[:], sd[:])
                    nmr = small.tile([128, 1], f32, tag="nmr", name="nmr")
                    nc.gpsimd.tensor_scalar(
                        out=nmr[:], in0=mv[:, 0:1], scalar1=rr[:, 0:1],
                        scalar2=-1.0, op0=ALU.mult, op1=ALU.mult,
                    )
                    ot = outs.tile([128, E2], bf16, tag="ot", name="ot")
                    if t >= NT - 3:
                        # drain the tail faster: split the normalize across
                        # scalar and vector for the last tiles.
                        nc.scalar.activation(
                            ot[:, 0:512], y[:, 0:512], AF.Identity,
                            bias=nmr[:, 0:1], scale=rr[:, 0:1],
                        )
                        nc.vector.tensor_scalar(
                            out=ot[:, 512:E2], in0=y[:, 512:E2],
                            scalar1=mv[:, 0:1], scalar2=rr[:, 0:1],
                            op0=ALU.subtract, op1=ALU.mult,
                        )
                    else:
                        nc.scalar.activation(
                            ot[:], y[:], AF.Identity,
                            bias=nmr[:, 0:1], scale=rr[:, 0:1],
                        )
                    if apply_ln_affine:
                        nc.vector.tensor_tensor(out=ot[:], in0=ot[:], in1=lnw_b[:], op=ALU.mult)
                        nc.vector.tensor_tensor(out=ot[:], in0=ot[:], in1=lnb_b[:], op=ALU.add)
                    nc.sync.dma_start(out=outd[tok, :], in_=ot[:])

    nc.compile()
    return nc


def _get_nc(apply_ln_affine: bool):
    key = ("nc", apply_ln_affine)
    if key not in _CACHE:
        _CACHE[key] = _build(apply_ln_affine)
    return _CACHE[key]


def kernel(x1, x2, conv_w, conv_b, ln_w, ln_b, _trace=False, _trace_kwargs=None):
    from concourse.bass_utils import run_bass_kernel_spmd
    import ml_dtypes

    bf16 = ml_dtypes.bfloat16

    x1 = np.asarray(x1, dtype=np.float32)
    x2 = np.ascontiguousarray(np.asarray(x2, dtype=np.float32))
    conv_w = np.asarray(conv_w, dtype=np.float32)
    conv_b = np.asarray(conv_b, dtype=np.float32)
    ln_w = np.asarray(ln_w, dtype=np.float32)
    ln_b = np.asarray(ln_b, dtype=np.float32)

    apply_affine = not (
        np.all(ln_w == 1.0) and np.all(ln_b == 0.0)
    )
    nc = _get_nc(apply_affine)

    convT = np.ascontiguousarray(conv_w.T.astype(bf16))  # [D, 2D]
    # the bias rides a ones[128,128] matmul (x128), so ship conv_b/1024
    cb8 = np.ascontiguousarray((conv_b / 1024.0).reshape(1, -1).astype(bf16))
    in_maps = []
    for b in range(B):
        xmix = np.empty((N, XW), dtype=bf16)
        xmix[:, 0:D] = x2[b].astype(bf16)
        x1h = x1[b].astype(bf16)
        for p in range(4):
            base = D + p * WIN
            xmix[:, base:base + 2] = 1.0
            xmix[:, base + 2:base + WIN] = x1h[:, p * 128:(p + 1) * 128]
        m = {
            "xmix": xmix,
            "convT": convT,
            "convb8": cb8,
        }
        if apply_affine:
            m["lnw"] = np.ascontiguousarray(ln_w.reshape(1, -1))
            m["lnb"] = np.ascontiguousarray(ln_b.reshape(1, -1))
        in_maps.append(m)

    kw = dict(_trace_kwargs or {})
    res = run_bass_kernel_spmd(nc, in_maps, list(range(B)), trace=_trace, **kw)
    out = np.stack([np.asarray(res.results[b]["out"], dtype=np.float32)
                    for b in range(B)], axis=0)
    if _trace:
        _CACHE["last_results"] = res
    return out
